# revision 10
# baseline (speedup 1.0000x reference)
"""Trainium2 Bass kernel for the Tsit5 neural-ODE solver (nn_NeuralODESolver).

Strategy (pure data parallel over batch, 8 cores):
  - Each core integrates a 512-row batch shard for num_steps Tsit5 steps.
  - Activations/weights in fp16 on the matmul path (PE streams 16-bit at
    1 row/cycle with overlapped weight loads); PSUM accumulates fp32.
  - Layout: feature-major ("transposed") — activations are [features, batch]
    so the batch is the matmul free dimension (N=512).
  - RK stage combinations y + dt*sum(a_ji k_i) are folded into layer-1 PSUM
    accumulation using host-prescaled copies of W1's x-columns, so almost no
    vector-engine AXPY work remains.
  - y_{t+1} = y_t + dt*sum(b_i k_i) is computed on the tensor engine with
    scaled-identity weights accumulating into a PSUM bank.
  - PSUM->SBUF evacuations (relu+bias) are split across ScalarE and VectorE
    to shorten the serial dependency chain.
All sharding/transposition/prescaling happens host-side in kernel().
"""

import numpy as np

import trn_fix

trn_fix.install()

import concourse.bass as bass  # noqa: E402
import concourse.tile as tile  # noqa: E402
from concourse import mybir  # noqa: E402
from concourse.bass_utils import run_bass_kernel_spmd  # noqa: E402

F32 = mybir.dt.float32
F16 = mybir.dt.float16
AT = mybir.ActivationFunctionType
OP = mybir.AluOpType

DT = 60.0 / 3600.0
A21 = 0.161
A31, A32 = -0.008480655492356989, 0.335480655492357
A41, A42, A43 = 2.8971530571054935, -6.359448489975075, 4.3622954328695815
A51, A52, A53, A54 = 5.325864828439257, -11.748883564062828, 7.4955393428898365, -0.09249506636175525
A61, A62, A63, A64, A65 = 5.86145544294642, -12.92096931784711, 8.159367898576159, -0.071584973281401, -0.028269050394068383
B1, B2, B3, B4, B5, B6 = 0.09646076681806523, 0.01, 0.4798896504144996, 1.379008574103742, -3.290069515436081, 2.324710524099774

N_CORES = 8
B, D, U, H = 4096, 64, 64, 256
NB = B // N_CORES  # 512 batch rows per core

# stage-fold groups: stage j uses kappa_1..kappa_{j-1}
# groups over the kappa-pair tiles k12, k34, k56 (pairs of 64-row blocks)
# each entry: (stage_key, rhs_pair_index, coeff_lo, coeff_hi_or_None)
FOLD_GROUPS = {
    3: [(0, A31, A32)],
    4: [(0, A41, A42), (1, A43, None)],
    5: [(0, A51, A52), (1, A53, A54)],
    6: [(0, A61, A62), (1, A63, A64), (2, A65, None)],
}


def _build_program(num_steps: int, split_waits: bool = True, unroll: int = 0):
    """unroll=0: fully unroll all steps. unroll=U>0: For_i loop whose body
    holds U steps (U must be even and divide num_steps)."""
    nc = bass.Bass("TRN2", target_bir_lowering=False, debug=False)

    # ---- DRAM I/O (per core shapes) ----
    base0_d = nc.dram_tensor("base0", [128, NB], F16, kind="ExternalInput")
    w1T_d = nc.dram_tensor("w1T", [128, H], F16, kind="ExternalInput")
    w2T0_d = nc.dram_tensor("w2T0", [128, H], F16, kind="ExternalInput")
    w2T1_d = nc.dram_tensor("w2T1", [128, H], F16, kind="ExternalInput")
    w3T0_d = nc.dram_tensor("w3T0", [128, D], F16, kind="ExternalInput")
    w3T1_d = nc.dram_tensor("w3T1", [128, D], F16, kind="ExternalInput")
    # fold weights (prescaled W1 x-columns), one per group
    fold_d = {}
    for j, groups in FOLD_GROUPS.items():
        for gi, (pi, clo, chi) in enumerate(groups):
            rows = 128 if chi is not None else 64
            fold_d[(j, gi)] = nc.dram_tensor(
                f"g{j}_{gi}", [rows, H], F16, kind="ExternalInput"
            )
    iy_d = nc.dram_tensor("iy", [64, 64], F16, kind="ExternalInput")
    i12_d = nc.dram_tensor("i12", [128, 64], F16, kind="ExternalInput")
    i34_d = nc.dram_tensor("i34", [128, 64], F16, kind="ExternalInput")
    i56_d = nc.dram_tensor("i56", [128, 64], F16, kind="ExternalInput")
    b1_d = nc.dram_tensor("b1c", [128, 2], F32, kind="ExternalInput")
    b2_d = nc.dram_tensor("b2c", [128, 2], F32, kind="ExternalInput")
    b3_d = nc.dram_tensor("b3c", [64, 1], F32, kind="ExternalInput")
    y0_d = nc.dram_tensor("y0f", [64, NB], F32, kind="ExternalInput")
    yT_d = nc.dram_tensor("yT", [64, NB], F32, kind="ExternalOutput")

    with tile.TileContext(nc) as tc:
        import contextlib

        ctx = contextlib.ExitStack()
        with ctx:
            const = ctx.enter_context(tc.tile_pool(name="const", bufs=1))
            act = ctx.enter_context(tc.tile_pool(name="act", bufs=2))
            pp = ctx.enter_context(tc.tile_pool(name="pp", bufs=1, space="PSUM"))
            ppk = ctx.enter_context(tc.tile_pool(name="ppk", bufs=2, space="PSUM"))

            def load(dram, shape, dtype, tag):
                t = const.tile(shape, dtype, tag=tag, name=tag)
                nc.sync.dma_start(t[:], dram[:, :])
                return t

            base = load(base0_d, [128, NB], F16, "base")
            xin2 = load(base0_d, [128, NB], F16, "xin2")  # rows 64: u, 0:64 rewritten
            w1T = load(w1T_d, [128, H], F16, "w1T")
            w2T0 = load(w2T0_d, [128, H], F16, "w2T0")
            w2T1 = load(w2T1_d, [128, H], F16, "w2T1")
            w3T0 = load(w3T0_d, [128, D], F16, "w3T0")
            w3T1 = load(w3T1_d, [128, D], F16, "w3T1")
            folds = {}
            for (j, gi), dram in fold_d.items():
                rows = dram.shape[0]
                folds[(j, gi)] = load(dram, [rows, H], F16, f"g{j}_{gi}")
            iy = load(iy_d, [64, 64], F16, "iy")
            ipair = [
                load(i12_d, [128, 64], F16, "i12"),
                load(i34_d, [128, 64], F16, "i34"),
                load(i56_d, [128, 64], F16, "i56"),
            ]
            b1c = load(b1_d, [128, 2], F32, "b1c")
            b2c = load(b2_d, [128, 2], F32, "b2c")
            b3c = load(b3_d, [64, 1], F32, "b3c")
            # fp32 master state (ping-pong pair)
            ym = [
                const.tile([64, NB], F32, tag=f"ym{i}", name=f"ym{i}")
                for i in range(2)
            ]
            nc.sync.dma_start(ym[0][:], y0_d[:, :])

            # persistent activation tiles
            kp = [
                const.tile([128, NB], F16, tag=f"k{p}", name=f"k{p}")
                for p in range(3)
            ]  # kappa pairs: k12, k34, k56
            h1 = [
                const.tile([128, NB], F16, tag=f"h1m{m}", name=f"h1m{m}")
                for m in range(2)
            ]
            h2 = [
                const.tile([128, NB], F16, tag=f"h2m{m}", name=f"h2m{m}")
                for m in range(2)
            ]

            SPLIT = 192  # ACT gets cols [0:SPLIT], DVE the rest

            def evac_split(dst_ap, src_ap, bias_ap, relu):
                # dst/src: [P, NB] APs; split columns across ACT and DVE
                if relu:
                    nc.scalar.activation(
                        dst_ap[:, 0:SPLIT], src_ap[:, 0:SPLIT], AT.Relu,
                        bias=bias_ap,
                    )
                    nc.vector.tensor_scalar(
                        dst_ap[:, SPLIT:NB], src_ap[:, SPLIT:NB],
                        bias_ap, 0.0, op0=OP.add, op1=OP.max,
                    )
                else:
                    nc.scalar.activation(
                        dst_ap[:, 0:SPLIT], src_ap[:, 0:SPLIT], AT.Identity,
                        bias=bias_ap,
                    )
                    nc.vector.tensor_scalar(
                        dst_ap[:, SPLIT:NB], src_ap[:, SPLIT:NB],
                        bias_ap, None, op0=OP.add,
                    )

            def evac_whole(dst_ap, src_ap, bias_ap, engine):
                if engine == "act":
                    nc.scalar.activation(dst_ap, src_ap, AT.Relu, bias=bias_ap)
                else:
                    nc.vector.tensor_scalar(
                        dst_ap, src_ap, bias_ap, 0.0, op0=OP.add, op1=OP.max
                    )

            def emit_step(step, parity, last):
                py = pp.tile([64, NB], F32, tag="py", name="py")
                ycur = ym[parity]
                ynext = ym[1 - parity]

                for j in range(1, 7):
                    # ---- L1 ----
                    ph1 = [
                        pp.tile([128, NB], F32, tag=f"ph1m{m}", name=f"ph1m{m}")
                        for m in range(2)
                    ]
                    for m in range(2):
                        mm = slice(m * 128, (m + 1) * 128)
                        rhs = xin2 if j == 2 else base
                        ngroups = 0 if j < 3 else len(FOLD_GROUPS[j])
                        nc.tensor.matmul(
                            ph1[m][:], w1T[:, mm], rhs[:],
                            start=True, stop=(ngroups == 0),
                        )
                        if ngroups:
                            for gi, (pi, clo, chi) in enumerate(FOLD_GROUPS[j]):
                                g = folds[(j, gi)]
                                rows = 128 if chi is not None else 64
                                nc.tensor.matmul(
                                    ph1[m][:], g[0:rows, mm], kp[pi][0:rows, :],
                                    start=False, stop=(gi == ngroups - 1),
                                )
                        evac_split(h1[m][:], ph1[m][:], b1c[:, m : m + 1], True)

                    # ---- L2 ----
                    ph2 = [
                        pp.tile([128, NB], F32, tag=f"ph2m{m}", name=f"ph2m{m}")
                        for m in range(2)
                    ]
                    for m in range(2):
                        mm = slice(m * 128, (m + 1) * 128)
                        nc.tensor.matmul(
                            ph2[m][:], w2T0[:, mm], h1[0][:], start=True, stop=False
                        )
                        nc.tensor.matmul(
                            ph2[m][:], w2T1[:, mm], h1[1][:], start=False, stop=True
                        )
                        evac_split(h2[m][:], ph2[m][:], b2c[:, m : m + 1], True)

                    # ---- L3 -> kappa_j ----
                    pk = ppk.tile([64, NB], F32, tag="pk", name="pk")
                    nc.tensor.matmul(pk[:], w3T0[:], h2[0][:], start=True, stop=False)
                    nc.tensor.matmul(pk[:], w3T1[:], h2[1][:], start=False, stop=True)
                    pi, lohi = divmod(j - 1, 2)
                    krows = kp[pi][64 * lohi : 64 * lohi + 64, :]
                    evac_split(krows, pk[:], b3c[:, 0:1], False)

                    if j == 1:
                        # xin2 = y + DT*A21*kappa1  (single DVE fused axpy)
                        nc.vector.scalar_tensor_tensor(
                            xin2[0:64, :], krows, DT * A21, ycur[:],
                            op0=OP.mult, op1=OP.add,
                        )
                    if j % 2 == 0:
                        # kappa pair pi complete -> accumulate y-increment
                        nc.tensor.matmul(
                            py[:], ipair[pi][:], kp[pi][:],
                            start=(j == 2), stop=(j == 6),
                        )

                # y_{t+1} = y_master + increment (fp32), then fp16 copy to base
                nc.vector.scalar_tensor_tensor(
                    ynext[:], py[:], 1.0, ycur[:], op0=OP.mult, op1=OP.add
                )
                if not last:
                    nc.scalar.activation(base[0:64, :], ynext[:], AT.Copy)

            if unroll == 0:
                for step in range(num_steps):
                    emit_step(step, step % 2, step == num_steps - 1)
                final_y = ym[num_steps % 2]
            else:
                assert unroll % 2 == 0 and num_steps % unroll == 0
                n_iters = num_steps // unroll
                with tc.For_i(0, n_iters, 1):
                    for s in range(unroll):
                        emit_step(s, s % 2, False)
                final_y = ym[0]
            nc.sync.dma_start(yT_d[:, :], final_y[:])

    if split_waits:
        trn_fix.split_excess_waits(nc)
    return nc


_PROGRAM_CACHE = {}


def _get_program(num_steps):
    if num_steps not in _PROGRAM_CACHE:
        _PROGRAM_CACHE[num_steps] = _build_program(num_steps)
    return _PROGRAM_CACHE[num_steps]


def _host_prep(x0, u, W1, b1, W2, b2, W3, b3):
    """Build per-core input maps (host-side sharding + layout)."""
    W1 = np.asarray(W1, np.float32)
    W2 = np.asarray(W2, np.float32)
    W3 = np.asarray(W3, np.float32)
    w1T = W1.T.astype(np.float16)  # [128, 256]
    w1xT = W1.T[0:D, :]  # fp32 x-part [64, 256]
    w2T0 = W2[:, 0:128].T.astype(np.float16)
    w2T1 = W2[:, 128:256].T.astype(np.float16)
    w3T0 = W3[:, 0:128].T.astype(np.float16)
    w3T1 = W3[:, 128:256].T.astype(np.float16)

    shared = {
        "w1T": w1T, "w2T0": w2T0, "w2T1": w2T1,
        "w3T0": w3T0, "w3T1": w3T1,
    }
    for j, groups in FOLD_GROUPS.items():
        for gi, (pi, clo, chi) in enumerate(groups):
            lo = (DT * clo) * w1xT
            if chi is not None:
                g = np.concatenate([lo, (DT * chi) * w1xT], axis=0)
            else:
                g = lo
            shared[f"g{j}_{gi}"] = g.astype(np.float16)
    eye = np.eye(64, dtype=np.float32)
    shared["iy"] = eye.astype(np.float16)
    for name, (ca, cb) in (
        ("i12", (B1, B2)), ("i34", (B3, B4)), ("i56", (B5, B6))
    ):
        m = np.concatenate([(DT * ca) * eye, (DT * cb) * eye], axis=0)
        shared[name] = m.astype(np.float16)
    shared["b1c"] = np.asarray(b1, np.float32).reshape(2, 128).T.copy()
    shared["b2c"] = np.asarray(b2, np.float32).reshape(2, 128).T.copy()
    shared["b3c"] = np.asarray(b3, np.float32).reshape(64, 1).copy()

    x0 = np.asarray(x0, np.float32)
    u = np.asarray(u, np.float32)
    in_maps = []
    for c in range(N_CORES):
        sl = slice(c * NB, (c + 1) * NB)
        basec = np.concatenate(
            [x0[sl].T, u[sl].T], axis=0
        ).astype(np.float16)  # [128, NB]
        m = dict(shared)
        m["base0"] = basec
        m["y0f"] = np.ascontiguousarray(x0[sl].T)  # [64, NB] fp32
        in_maps.append(m)
    return in_maps


def kernel(x0, u, W1, b1, W2, b2, W3, b3, num_steps):
    steps = int(num_steps)
    nc = _get_program(steps)
    in_maps = _host_prep(x0, u, W1, b1, W2, b2, W3, b3)
    res = run_bass_kernel_spmd(nc, in_maps, list(range(N_CORES)))
    out = np.empty((B, D), np.float32)
    for c in range(N_CORES):
        out[c * NB : (c + 1) * NB] = res.results[c]["yT"].T
    return out
